# revision 19
# baseline (speedup 1.0000x reference)
"""Trainium2 Bass kernel for MoE routing (nn_MoE_74071005987155).

Computes: logits = x @ W.T + b; probs = softmax(logits, axis=-1);
vals, idx = top_k(probs.T, k=1024)  -> ([64, 1024] f32, [64, 1024] i32)

Distribution: x token-sharded across 8 cores (4096 tokens each), W/b
replicated.  Each core computes probsT for its tokens (PE transposes +
fp32 matmul + free-axis softmax), an AllToAll exchanges probsT so core c
owns experts [8c, 8c+8) over all 32768 tokens, then 4 rounds of the
gpsimd top-256 ucode (k=256, vocab=50176) with per-expert threshold
masking between rounds produce the exact sorted top-1024 per expert.
Host only shards inputs and reassembles/reorders the raw round dumps.
"""

import numpy as np

NCORES = 8
T = 32768
D = 2048
E = 64
K = 1024
TSH = T // NCORES          # 4096 tokens per core
EPC = E // NCORES          # 8 experts per core
VPP = 3136                 # vocab elements per partition (50176 / 16)
VOCAB = 16 * VPP           # 50176 (> 50000 required by the topk ucode)
ROUNDS = 4                 # 4 x 256 = 1024
TAIL = TSH - VPP           # 960 tokens on the odd partition of each core pair
NEG = -1.0e30

_CACHED = {}


def _build_nc():
    from concourse import bacc, mybir, tile

    f32 = mybir.dt.float32
    u32 = mybir.dt.uint32
    Alu = mybir.AluOpType
    Act = mybir.ActivationFunctionType

    nc = bacc.Bacc("TRN2", target_bir_lowering=False, debug=False,
                   num_devices=NCORES)

    # x shard arrives d-major ([D, TSH]) so matmul operands stream directly
    x_d = nc.dram_tensor("x", [D, TSH], f32, kind="ExternalInput").ap()
    w_d = nc.dram_tensor("W", [E, D], f32, kind="ExternalInput").ap()
    b_d = nc.dram_tensor("b", [E, 1], f32, kind="ExternalInput").ap()
    ident_d = nc.dram_tensor("ident", [128, 128], f32, kind="ExternalInput").ap()
    raw_d = nc.dram_tensor("raw", [ROUNDS, 128, 32], u32, kind="ExternalOutput").ap()

    # all-to-all staged per quarter (2 blocks = 1024 tokens) for overlap
    NQ = 4
    QT = TSH // NQ  # 1024 tokens per quarter
    pt_drams = [nc.dram_tensor(f"pt_stage{q}", [E, QT], f32).ap()
                for q in range(NQ)]
    a2a_drams = [nc.dram_tensor(f"a2a_out{q}", [E, QT], f32).ap()
                 for q in range(NQ)]

    with tile.TileContext(nc) as tc:
        with (
            tc.tile_pool(name="consts", bufs=1) as consts,
            tc.tile_pool(name="xts", bufs=6) as xts_pool,
            tc.tile_pool(name="soft", bufs=2) as soft_pool,
            tc.tile_pool(name="ptile", bufs=2) as pt_pool,
            tc.tile_pool(name="xtp", bufs=2, space="PSUM") as xtp_pool,
            tc.tile_pool(name="ltp", bufs=3, space="PSUM") as lt_pool,
        ):
            # ---- constants ----
            ident = consts.tile([128, 128], f32)
            nc.sync.dma_start(out=ident[:, :], in_=ident_d[:, :])
            b_sb = consts.tile([E, 1], f32)
            nc.sync.dma_start(out=b_sb[:, :], in_=b_d[:, :])

            # ---- WT: [128 d-chunk, 64 e] x 16 chunks, from W [64, 2048] ----
            w_sb = consts.tile([E, D], f32)
            nc.sync.dma_start(out=w_sb[:, :], in_=w_d[:, :])
            wt_sb = consts.tile([128, 16 * E], f32)
            for c in range(16):
                wtp = xtp_pool.tile([128, 512], f32, tag="xtp")
                nc.tensor.transpose(wtp[:, 0:E], w_sb[:, 128 * c:128 * c + 128],
                                    ident[0:E, 0:E])
                nc.scalar.activation(wt_sb[:, E * c:E * c + E], wtp[:, 0:E],
                                     Act.Copy)

            # ---- G buffer (topk input), padded with -1e30 ----
            g = nc.alloc_sbuf_tensor("g_sb", [128, VPP], f32).ap()
            nc.vector.memset(g[:, :], NEG)

            # ---- main loop: 8 blocks x 512 tokens ----
            for blk in range(8):
                lt = lt_pool.tile([E, 512], f32, tag="ltp")
                for c in range(16):
                    xts = xts_pool.tile([128, 512], f32, tag="xts")
                    nc.sync.dma_start(
                        out=xts[:, :],
                        in_=x_d[128 * c:128 * c + 128,
                                512 * blk:512 * blk + 512])
                    nc.tensor.matmul(lt[:, :], wt_sb[:, E * c:E * c + E],
                                     xts[:, :], start=(c == 0), stop=(c == 15))

                # bias add (b may be nonzero in general) -> SBUF
                lt_sb = soft_pool.tile([E, 512], f32, tag="lt_sb")
                nc.vector.tensor_scalar(out=lt_sb[:, :], in0=lt[:, :],
                                        scalar1=b_sb[:, :], scalar2=None,
                                        op0=Alu.add)

                # softmax over experts = partition axis of logitsT [64, 512]
                from concourse import bass_isa
                mx = soft_pool.tile([E, 512], f32, tag="mx")
                nc.gpsimd.partition_all_reduce(mx[:, :], lt_sb[:, :],
                                               channels=E,
                                               reduce_op=bass_isa.ReduceOp.max)
                sub = soft_pool.tile([E, 512], f32, tag="sub")
                nc.vector.tensor_tensor(out=sub[:, :], in0=lt_sb[:, :],
                                        in1=mx[:, :], op=Alu.subtract)
                ex = soft_pool.tile([E, 512], f32, tag="ex")
                nc.scalar.activation(ex[:, :], sub[:, :], Act.Exp)
                sm = soft_pool.tile([E, 512], f32, tag="sm")
                nc.gpsimd.partition_all_reduce(sm[:, :], ex[:, :],
                                               channels=E,
                                               reduce_op=bass_isa.ReduceOp.add)
                rec = soft_pool.tile([E, 512], f32, tag="rec")
                nc.vector.reciprocal(out=rec[:, :], in_=sm[:, :])
                pts = pt_pool.tile([E, 512], f32, tag="pts")
                nc.vector.tensor_tensor(out=pts[:, :], in0=ex[:, :],
                                        in1=rec[:, :], op=Alu.mult)
                q, half = blk // 2, blk % 2
                nc.scalar.dma_start(
                    out=pt_drams[q][:, 512 * half:512 * half + 512],
                    in_=pts[:, :])

                # after each odd block: all-to-all this quarter and fill G.
                # G layout: expert el -> partitions 16el..16el+15; partition
                # 16el+i (i<8) = core i tokens [0, VPP); 16el+8+i = core i
                # tokens [VPP, TSH) plus pad.
                if half == 1:
                    nc.gpsimd.collective_compute(
                        "AllToAll", mybir.AluOpType.bypass,
                        replica_groups=[list(range(NCORES))],
                        ins=[pt_drams[q][:, :]], outs=[a2a_drams[q][:, :]])
                    a2a_r = a2a_drams[q].rearrange("(i e) t -> e i t", e=EPC)
                    c0 = QT * q          # global token offset of this quarter
                    for el in range(EPC):
                        if c0 + QT <= VPP:
                            nc.scalar.dma_start(
                                out=g[16 * el:16 * el + 8, c0:c0 + QT],
                                in_=a2a_r[el][:, :])
                        else:
                            head = VPP - c0
                            nc.scalar.dma_start(
                                out=g[16 * el:16 * el + 8, c0:VPP],
                                in_=a2a_r[el][:, 0:head])
                            nc.scalar.dma_start(
                                out=g[16 * el + 8:16 * el + 16, 0:QT - head],
                                in_=a2a_r[el][:, head:QT])

            # ---- 4 rounds of top-256 with threshold masking ----
            for r in range(ROUNDS):
                o = nc.alloc_sbuf_tensor(f"tk{r}", [128, 32], u32).ap()
                nc.gpsimd.topk(o[:, :], g[:, :], tokens=EPC,
                               vocab_size=VOCAB, k=256)
                nc.sync.dma_start(out=raw_d[r], in_=o[:, :])
                if r < ROUNDS - 1:
                    tb = soft_pool.tile([128, 1], f32, tag=f"tb{r}")
                    nc.vector.stream_shuffle(out=tb[:, :],
                                             in_=o[:, 0:1].bitcast(f32),
                                             mask=[0] * 16 + [16] * 16)
                    nc.vector.scalar_tensor_tensor(
                        out=g[:, :], in0=g[:, :], scalar=tb[:, :],
                        in1=g[:, :], op0=Alu.is_lt, op1=Alu.mult)

    nc.compile()
    return nc


def _get_nc():
    if "nc" not in _CACHED:
        _CACHED["nc"] = _build_nc()
    return _CACHED["nc"]


def _decode_flat_idx(flat):
    """ucode flat index within an expert row -> global token id.

    Partition p (of 16): p < 8 -> core p, token offset o; p >= 8 ->
    core p-8, token offset VPP + o (the TAIL chunk).
    """
    p = flat // VPP
    o = flat - p * VPP
    core = np.where(p < 8, p, p - 8)
    base = np.where(p < 8, 0, VPP)
    return TSH * core + base + o


def make_in_maps(x, W, b):
    x = np.ascontiguousarray(np.asarray(x), dtype=np.float32)
    W = np.ascontiguousarray(np.asarray(W), dtype=np.float32)
    b = np.ascontiguousarray(np.asarray(b), dtype=np.float32).reshape(E, 1)
    assert x.shape == (T, D) and W.shape == (E, D)
    ident = np.eye(128, dtype=np.float32)
    return [
        {"x": np.ascontiguousarray(x[TSH * c:TSH * (c + 1)].T), "W": W,
         "b": b, "ident": ident}
        for c in range(NCORES)
    ]


def kernel(x, W, b, k):
    from concourse.bass_utils import run_bass_kernel_spmd

    assert int(k) == K, f"kernel compiled for k={K}, got {k}"
    in_maps = make_in_maps(x, W, b)
    nc = _get_nc()
    res = run_bass_kernel_spmd(nc, in_maps, core_ids=list(range(NCORES)))

    vals = np.empty((E, K), dtype=np.float32)
    idx = np.empty((E, K), dtype=np.int32)
    for c in range(NCORES):
        raw = res.results[c]["raw"]          # [ROUNDS, 128, 32] uint32
        for el in range(EPC):
            e = EPC * c + el
            blk = raw[:, 16 * el:16 * el + 16, :]        # [ROUNDS, 16, 32]
            v = blk[:, :, :16].reshape(ROUNDS, 256).view(np.float32)[:, ::-1]
            fi = blk[:, :, 16:32].reshape(ROUNDS, 256)[:, ::-1]
            vals[e] = v.reshape(K)
            idx[e] = _decode_flat_idx(fi.astype(np.int64)).reshape(K)
    return vals, idx


# revision 20
# speedup vs baseline: 1.0508x; 1.0508x over previous
"""Trainium2 Bass kernel for MoE routing (nn_MoE_74071005987155).

Computes: logits = x @ W.T + b; probs = softmax(logits, axis=-1);
vals, idx = top_k(probs.T, k=1024)  -> ([64, 1024] f32, [64, 1024] i32)

Distribution: x token-sharded across 8 cores (4096 tokens each), W/b
replicated.  Each core computes probsT for its tokens (PE transposes +
fp32 matmul + free-axis softmax), an AllToAll exchanges probsT so core c
owns experts [8c, 8c+8) over all 32768 tokens, then 4 rounds of the
gpsimd top-256 ucode (k=256, vocab=50176) with per-expert threshold
masking between rounds produce the exact sorted top-1024 per expert.
Host only shards inputs and reassembles/reorders the raw round dumps.
"""

import numpy as np

NCORES = 8
T = 32768
D = 2048
E = 64
K = 1024
TSH = T // NCORES          # 4096 tokens per core
EPC = E // NCORES          # 8 experts per core
VPP = 3136                 # vocab elements per partition (50176 / 16)
VOCAB = 16 * VPP           # 50176 (> 50000 required by the topk ucode)
ROUNDS = 4                 # 4 x 256 = 1024
TAIL = TSH - VPP           # 960 tokens on the odd partition of each core pair
NEG = -1.0e30

_CACHED = {}


def _build_nc():
    from concourse import bacc, mybir, tile

    f32 = mybir.dt.float32
    u32 = mybir.dt.uint32
    Alu = mybir.AluOpType
    Act = mybir.ActivationFunctionType

    nc = bacc.Bacc("TRN2", target_bir_lowering=False, debug=False,
                   num_devices=NCORES)

    # x shard arrives d-major ([D, TSH]) so matmul operands stream directly
    x_d = nc.dram_tensor("x", [D, TSH], f32, kind="ExternalInput").ap()
    w_d = nc.dram_tensor("W", [E, D], f32, kind="ExternalInput").ap()
    b_d = nc.dram_tensor("b", [E, 1], f32, kind="ExternalInput").ap()
    ident_d = nc.dram_tensor("ident", [128, 128], f32, kind="ExternalInput").ap()
    raw_d = nc.dram_tensor("raw", [ROUNDS, 128, 32], u32, kind="ExternalOutput").ap()

    # all-to-all staged per quarter (2 blocks = 1024 tokens) for overlap
    NQ = 4
    QT = TSH // NQ  # 1024 tokens per quarter
    pt_drams = [nc.dram_tensor(f"pt_stage{q}", [E, QT], f32).ap()
                for q in range(NQ)]
    a2a_drams = [nc.dram_tensor(f"a2a_out{q}", [E, QT], f32).ap()
                 for q in range(NQ)]

    with tile.TileContext(nc) as tc:
        with (
            tc.tile_pool(name="consts", bufs=1) as consts,
            tc.tile_pool(name="xts", bufs=6) as xts_pool,
            tc.tile_pool(name="soft", bufs=2) as soft_pool,
            tc.tile_pool(name="ptile", bufs=2) as pt_pool,
            tc.tile_pool(name="xtp", bufs=2, space="PSUM") as xtp_pool,
            tc.tile_pool(name="ltp", bufs=2, space="PSUM") as lt_pool,
            tc.tile_pool(name="lgp", bufs=2, space="PSUM") as lg_pool,
            tc.tile_pool(name="ptp", bufs=2, space="PSUM") as ptp_pool,
        ):
            # ---- constants ----
            ident = consts.tile([128, 128], f32)
            nc.sync.dma_start(out=ident[:, :], in_=ident_d[:, :])
            b_sb = consts.tile([E, 1], f32)
            nc.sync.dma_start(out=b_sb[:, :], in_=b_d[:, :])

            # ---- WT: [128 d-chunk, 64 e] x 16 chunks, from W [64, 2048] ----
            w_sb = consts.tile([E, D], f32)
            nc.sync.dma_start(out=w_sb[:, :], in_=w_d[:, :])
            wt_sb = consts.tile([128, 16 * E], f32)
            for c in range(16):
                wtp = xtp_pool.tile([128, 512], f32, tag="xtp")
                nc.tensor.transpose(wtp[:, 0:E], w_sb[:, 128 * c:128 * c + 128],
                                    ident[0:E, 0:E])
                nc.scalar.activation(wt_sb[:, E * c:E * c + E], wtp[:, 0:E],
                                     Act.Copy)

            # ---- G buffer (topk input), padded with -1e30 ----
            g = nc.alloc_sbuf_tensor("g_sb", [128, VPP], f32).ap()
            nc.vector.memset(g[:, :], NEG)
            # warm-up call on the freshly-padded G: runs while the PE/DMA
            # phase is busy and gpsimd is otherwise idle
            o_warm = nc.alloc_sbuf_tensor("tk_warm", [128, 32], u32).ap()
            nc.gpsimd.topk(o_warm[:, :], g[:, :], tokens=EPC,
                           vocab_size=VOCAB, k=256)

            # ---- main loop: 8 blocks x 512 tokens ----
            for blk in range(8):
                lt = lt_pool.tile([E, 512], f32, tag="ltp")
                for c in range(16):
                    xts = xts_pool.tile([128, 512], f32, tag="xts")
                    nc.sync.dma_start(
                        out=xts[:, :],
                        in_=x_d[128 * c:128 * c + 128,
                                512 * blk:512 * blk + 512])
                    nc.tensor.matmul(lt[:, :], wt_sb[:, E * c:E * c + E],
                                     xts[:, :], start=(c == 0), stop=(c == 15))

                # bias add (b may be nonzero in general) -> SBUF
                lt_sb = soft_pool.tile([E, 512], f32, tag="lt_sb")
                nc.vector.tensor_scalar(out=lt_sb[:, :], in0=lt[:, :],
                                        scalar1=b_sb[:, :], scalar2=None,
                                        op0=Alu.add)

                # transpose logitsT -> logits [128 t, 4 j, 64 e] in PSUM
                lg = lg_pool.tile([128, 4 * E], f32, tag="lgp")
                for j in range(4):
                    nc.tensor.transpose(
                        lg[:, E * j:E * j + E],
                        lt_sb[:, 128 * j:128 * j + 128],
                        ident[0:E, 0:E])

                # softmax over experts (free axis), per 128-token chunk j
                negm = soft_pool.tile([128, 4], f32, tag="negm")
                nc.vector.tensor_reduce(
                    out=negm[:, :],
                    in_=lg[:, :].rearrange("p (j e) -> p j e", j=4),
                    axis=mybir.AxisListType.X, op=Alu.max, negate=True)
                ex = soft_pool.tile([128, 4 * E], f32, tag="ex")
                ssum = soft_pool.tile([128, 4], f32, tag="ssum")
                for j in range(4):
                    nc.scalar.activation(
                        ex[:, E * j:E * j + E], lg[:, E * j:E * j + E],
                        Act.Exp, bias=negm[:, j:j + 1], scale=1.0,
                        accum_out=ssum[:, j:j + 1])
                rec = soft_pool.tile([128, 4], f32, tag="rec")
                nc.vector.reciprocal(out=rec[:, :], in_=ssum[:, :])
                pr = soft_pool.tile([128, 4 * E], f32, tag="pr")
                for j in range(4):
                    nc.scalar.activation(
                        pr[:, E * j:E * j + E], ex[:, E * j:E * j + E],
                        Act.Copy, scale=rec[:, j:j + 1])

                # transpose probs back -> probsT [64, 512] and store
                ptp = ptp_pool.tile([E, 512], f32, tag="ptp")
                for j in range(4):
                    nc.tensor.transpose(
                        ptp[:, 128 * j:128 * j + 128],
                        pr[:, E * j:E * j + E],
                        ident[:, :])
                pts = pt_pool.tile([E, 512], f32, tag="pts")
                nc.vector.tensor_copy(out=pts[:, :], in_=ptp[:, :])
                q, half = blk // 2, blk % 2
                nc.scalar.dma_start(
                    out=pt_drams[q][:, 512 * half:512 * half + 512],
                    in_=pts[:, :])

                # after each odd block: all-to-all this quarter and fill G.
                # G layout: expert el -> partitions 16el..16el+15; partition
                # 16el+i (i<8) = core i tokens [0, VPP); 16el+8+i = core i
                # tokens [VPP, TSH) plus pad.
                if half == 1:
                    nc.gpsimd.collective_compute(
                        "AllToAll", mybir.AluOpType.bypass,
                        replica_groups=[list(range(NCORES))],
                        ins=[pt_drams[q][:, :]], outs=[a2a_drams[q][:, :]])
                    a2a_r = a2a_drams[q].rearrange("(i e) t -> e i t", e=EPC)
                    c0 = QT * q          # global token offset of this quarter
                    for el in range(EPC):
                        if c0 + QT <= VPP:
                            nc.scalar.dma_start(
                                out=g[16 * el:16 * el + 8, c0:c0 + QT],
                                in_=a2a_r[el][:, :])
                        else:
                            head = VPP - c0
                            nc.scalar.dma_start(
                                out=g[16 * el:16 * el + 8, c0:VPP],
                                in_=a2a_r[el][:, 0:head])
                            nc.scalar.dma_start(
                                out=g[16 * el + 8:16 * el + 16, 0:QT - head],
                                in_=a2a_r[el][:, head:QT])

            # ---- 4 rounds of top-256 with threshold masking ----
            for r in range(ROUNDS):
                o = nc.alloc_sbuf_tensor(f"tk{r}", [128, 32], u32).ap()
                nc.gpsimd.topk(o[:, :], g[:, :], tokens=EPC,
                               vocab_size=VOCAB, k=256)
                nc.sync.dma_start(out=raw_d[r], in_=o[:, :])
                if r < ROUNDS - 1:
                    tb = soft_pool.tile([128, 1], f32, tag=f"tb{r}")
                    nc.vector.stream_shuffle(out=tb[:, :],
                                             in_=o[:, 0:1].bitcast(f32),
                                             mask=[0] * 16 + [16] * 16)
                    nc.vector.scalar_tensor_tensor(
                        out=g[:, :], in0=g[:, :], scalar=tb[:, :],
                        in1=g[:, :], op0=Alu.is_lt, op1=Alu.mult)

    nc.compile()
    return nc


def _get_nc():
    if "nc" not in _CACHED:
        _CACHED["nc"] = _build_nc()
    return _CACHED["nc"]


def _decode_flat_idx(flat):
    """ucode flat index within an expert row -> global token id.

    Partition p (of 16): p < 8 -> core p, token offset o; p >= 8 ->
    core p-8, token offset VPP + o (the TAIL chunk).
    """
    p = flat // VPP
    o = flat - p * VPP
    core = np.where(p < 8, p, p - 8)
    base = np.where(p < 8, 0, VPP)
    return TSH * core + base + o


def make_in_maps(x, W, b):
    x = np.ascontiguousarray(np.asarray(x), dtype=np.float32)
    W = np.ascontiguousarray(np.asarray(W), dtype=np.float32)
    b = np.ascontiguousarray(np.asarray(b), dtype=np.float32).reshape(E, 1)
    assert x.shape == (T, D) and W.shape == (E, D)
    ident = np.eye(128, dtype=np.float32)
    return [
        {"x": np.ascontiguousarray(x[TSH * c:TSH * (c + 1)].T), "W": W,
         "b": b, "ident": ident}
        for c in range(NCORES)
    ]


def kernel(x, W, b, k):
    from concourse.bass_utils import run_bass_kernel_spmd

    assert int(k) == K, f"kernel compiled for k={K}, got {k}"
    in_maps = make_in_maps(x, W, b)
    nc = _get_nc()
    res = run_bass_kernel_spmd(nc, in_maps, core_ids=list(range(NCORES)))

    vals = np.empty((E, K), dtype=np.float32)
    idx = np.empty((E, K), dtype=np.int32)
    for c in range(NCORES):
        raw = res.results[c]["raw"]          # [ROUNDS, 128, 32] uint32
        for el in range(EPC):
            e = EPC * c + el
            blk = raw[:, 16 * el:16 * el + 16, :]        # [ROUNDS, 16, 32]
            v = blk[:, :, :16].reshape(ROUNDS, 256).view(np.float32)[:, ::-1]
            fi = blk[:, :, 16:32].reshape(ROUNDS, 256)[:, ::-1]
            vals[e] = v.reshape(K)
            idx[e] = _decode_flat_idx(fi.astype(np.int64)).reshape(K)
    return vals, idx


# revision 21
# speedup vs baseline: 1.0633x; 1.0119x over previous
"""Trainium2 Bass kernel for MoE routing (nn_MoE_74071005987155).

Computes: logits = x @ W.T + b; probs = softmax(logits, axis=-1);
vals, idx = top_k(probs.T, k=1024)  -> ([64, 1024] f32, [64, 1024] i32)

Distribution: x token-sharded across 8 cores (4096 tokens each), W/b
replicated.  Each core computes probsT for its tokens (PE transposes +
fp32 matmul + free-axis softmax), an AllToAll exchanges probsT so core c
owns experts [8c, 8c+8) over all 32768 tokens, then 4 rounds of the
gpsimd top-256 ucode (k=256, vocab=50176) with per-expert threshold
masking between rounds produce the exact sorted top-1024 per expert.
Host only shards inputs and reassembles/reorders the raw round dumps.
"""

import numpy as np

NCORES = 8
T = 32768
D = 2048
E = 64
K = 1024
TSH = T // NCORES          # 4096 tokens per core
EPC = E // NCORES          # 8 experts per core
VPP = 3136                 # vocab elements per partition (50176 / 16)
VOCAB = 16 * VPP           # 50176 (> 50000 required by the topk ucode)
ROUNDS = 4                 # 4 x 256 = 1024
TAIL = TSH - VPP           # 960 tokens on the odd partition of each core pair
NEG = -1.0e30

_CACHED = {}


def _build_nc():
    from concourse import bacc, mybir, tile

    f32 = mybir.dt.float32
    u32 = mybir.dt.uint32
    Alu = mybir.AluOpType
    Act = mybir.ActivationFunctionType

    nc = bacc.Bacc("TRN2", target_bir_lowering=False, debug=False,
                   num_devices=NCORES)

    # x shard arrives d-major ([D, TSH]) so matmul operands stream directly
    x_d = nc.dram_tensor("x", [D, TSH], f32, kind="ExternalInput").ap()
    w_d = nc.dram_tensor("W", [E, D], f32, kind="ExternalInput").ap()
    b_d = nc.dram_tensor("b", [E, 1], f32, kind="ExternalInput").ap()
    ident_d = nc.dram_tensor("ident", [128, 128], f32, kind="ExternalInput").ap()
    raw_d = nc.dram_tensor("raw", [ROUNDS, 128, 32], u32, kind="ExternalOutput").ap()

    # all-to-all staged unevenly for overlap: the last stage is a single
    # 512-token block so the post-matmul tail is as short as possible
    STAGE_BLOCKS = [(0, 3), (3, 5), (5, 7), (7, 8)]   # block ranges
    pt_drams = [nc.dram_tensor(f"pt_stage{q}", [E, 512 * (b1 - b0)], f32).ap()
                for q, (b0, b1) in enumerate(STAGE_BLOCKS)]
    a2a_drams = [nc.dram_tensor(f"a2a_out{q}", [E, 512 * (b1 - b0)], f32).ap()
                 for q, (b0, b1) in enumerate(STAGE_BLOCKS)]

    with tile.TileContext(nc) as tc:
        with (
            tc.tile_pool(name="consts", bufs=1) as consts,
            tc.tile_pool(name="xts", bufs=8) as xts_pool,
            tc.tile_pool(name="soft", bufs=2) as soft_pool,
            tc.tile_pool(name="ptile", bufs=2) as pt_pool,
            tc.tile_pool(name="xtp", bufs=2, space="PSUM") as xtp_pool,
            tc.tile_pool(name="ltp", bufs=2, space="PSUM") as lt_pool,
            tc.tile_pool(name="lgp", bufs=2, space="PSUM") as lg_pool,
            tc.tile_pool(name="ptp", bufs=2, space="PSUM") as ptp_pool,
        ):
            # ---- constants ----
            ident = consts.tile([128, 128], f32)
            nc.scalar.dma_start(out=ident[:, :], in_=ident_d[:, :])
            b_sb = consts.tile([E, 1], f32)
            nc.scalar.dma_start(out=b_sb[:, :], in_=b_d[:, :])

            # ---- WT: [128 d-chunk, 64 e] x 16 chunks, from W [64, 2048] ----
            w_sb = consts.tile([E, D], f32)
            nc.scalar.dma_start(out=w_sb[:, :], in_=w_d[:, :])
            wt_sb = consts.tile([128, 16 * E], f32)
            for c in range(16):
                wtp = xtp_pool.tile([128, 512], f32, tag="xtp")
                nc.tensor.transpose(wtp[:, 0:E], w_sb[:, 128 * c:128 * c + 128],
                                    ident[0:E, 0:E])
                nc.scalar.activation(wt_sb[:, E * c:E * c + E], wtp[:, 0:E],
                                     Act.Copy)

            # ---- G buffer (topk input), padded with -1e30 ----
            g = nc.alloc_sbuf_tensor("g_sb", [128, VPP], f32).ap()
            nc.vector.memset(g[:, :], NEG)
            # warm-up call on the freshly-padded G: runs while the PE/DMA
            # phase is busy and gpsimd is otherwise idle
            o_warm = nc.alloc_sbuf_tensor("tk_warm", [128, 32], u32).ap()
            nc.gpsimd.topk(o_warm[:, :], g[:, :], tokens=EPC,
                           vocab_size=VOCAB, k=256)

            # ---- main loop: 8 blocks x 512 tokens ----
            for blk in range(8):
                lt = lt_pool.tile([E, 512], f32, tag="ltp")
                for c in range(16):
                    xts = xts_pool.tile([128, 512], f32, tag="xts")
                    nc.sync.dma_start(
                        out=xts[:, :],
                        in_=x_d[128 * c:128 * c + 128,
                                512 * blk:512 * blk + 512])
                    nc.tensor.matmul(lt[:, :], wt_sb[:, E * c:E * c + E],
                                     xts[:, :], start=(c == 0), stop=(c == 15))

                # bias add (b may be nonzero in general) -> SBUF
                lt_sb = soft_pool.tile([E, 512], f32, tag="lt_sb")
                nc.vector.tensor_scalar(out=lt_sb[:, :], in0=lt[:, :],
                                        scalar1=b_sb[:, :], scalar2=None,
                                        op0=Alu.add)

                # transpose logitsT -> logits [128 t, 4 j, 64 e] in PSUM
                lg = lg_pool.tile([128, 4 * E], f32, tag="lgp")
                for j in range(4):
                    nc.tensor.transpose(
                        lg[:, E * j:E * j + E],
                        lt_sb[:, 128 * j:128 * j + 128],
                        ident[0:E, 0:E])

                # softmax over experts (free axis), per 128-token chunk j
                negm = soft_pool.tile([128, 4], f32, tag="negm")
                nc.vector.tensor_reduce(
                    out=negm[:, :],
                    in_=lg[:, :].rearrange("p (j e) -> p j e", j=4),
                    axis=mybir.AxisListType.X, op=Alu.max, negate=True)
                ex = soft_pool.tile([128, 4 * E], f32, tag="ex")
                ssum = soft_pool.tile([128, 4], f32, tag="ssum")
                for j in range(4):
                    nc.scalar.activation(
                        ex[:, E * j:E * j + E], lg[:, E * j:E * j + E],
                        Act.Exp, bias=negm[:, j:j + 1], scale=1.0,
                        accum_out=ssum[:, j:j + 1])
                rec = soft_pool.tile([128, 4], f32, tag="rec")
                nc.vector.reciprocal(out=rec[:, :], in_=ssum[:, :])
                pr = soft_pool.tile([128, 4 * E], f32, tag="pr")
                for j in range(4):
                    nc.scalar.activation(
                        pr[:, E * j:E * j + E], ex[:, E * j:E * j + E],
                        Act.Copy, scale=rec[:, j:j + 1])

                # transpose probs back -> probsT [64, 512] and store
                ptp = ptp_pool.tile([E, 512], f32, tag="ptp")
                for j in range(4):
                    nc.tensor.transpose(
                        ptp[:, 128 * j:128 * j + 128],
                        pr[:, E * j:E * j + E],
                        ident[:, :])
                pts = pt_pool.tile([E, 512], f32, tag="pts")
                nc.vector.tensor_copy(out=pts[:, :], in_=ptp[:, :])
                q = next(i for i, (b0, b1) in enumerate(STAGE_BLOCKS)
                         if b0 <= blk < b1)
                sb0, sb1 = STAGE_BLOCKS[q]
                nc.scalar.dma_start(
                    out=pt_drams[q][:, 512 * (blk - sb0):512 * (blk - sb0 + 512 // 512)],
                    in_=pts[:, :]) if False else nc.scalar.dma_start(
                    out=pt_drams[q][:, 512 * (blk - sb0):512 * (blk - sb0) + 512],
                    in_=pts[:, :])

                # at the last block of a stage: all-to-all it and fill G.
                # G layout: expert el -> partitions 16el..16el+15; partition
                # 16el+i (i<8) = core i tokens [0, VPP); 16el+8+i = core i
                # tokens [VPP, TSH) plus pad.
                if blk == sb1 - 1:
                    nc.gpsimd.collective_compute(
                        "AllToAll", mybir.AluOpType.bypass,
                        replica_groups=[list(range(NCORES))],
                        ins=[pt_drams[q][:, :]], outs=[a2a_drams[q][:, :]])
                    a2a_r = a2a_drams[q].rearrange("(i e) t -> e i t", e=EPC)
                    c0 = 512 * sb0       # global token offset of this stage
                    QT = 512 * (sb1 - sb0)
                    for el in range(EPC):
                        if c0 + QT <= VPP:
                            nc.scalar.dma_start(
                                out=g[16 * el:16 * el + 8, c0:c0 + QT],
                                in_=a2a_r[el][:, :])
                        elif c0 >= VPP:
                            nc.scalar.dma_start(
                                out=g[16 * el + 8:16 * el + 16,
                                      c0 - VPP:c0 - VPP + QT],
                                in_=a2a_r[el][:, :])
                        else:
                            head = VPP - c0
                            nc.scalar.dma_start(
                                out=g[16 * el:16 * el + 8, c0:VPP],
                                in_=a2a_r[el][:, 0:head])
                            nc.scalar.dma_start(
                                out=g[16 * el + 8:16 * el + 16, 0:QT - head],
                                in_=a2a_r[el][:, head:QT])

            # ---- 4 rounds of top-256 with threshold masking ----
            for r in range(ROUNDS):
                o = nc.alloc_sbuf_tensor(f"tk{r}", [128, 32], u32).ap()
                nc.gpsimd.topk(o[:, :], g[:, :], tokens=EPC,
                               vocab_size=VOCAB, k=256)
                nc.sync.dma_start(out=raw_d[r], in_=o[:, :])
                if r < ROUNDS - 1:
                    tb = soft_pool.tile([128, 1], f32, tag=f"tb{r}")
                    nc.vector.stream_shuffle(out=tb[:, :],
                                             in_=o[:, 0:1].bitcast(f32),
                                             mask=[0] * 16 + [16] * 16)
                    nc.vector.scalar_tensor_tensor(
                        out=g[:, :], in0=g[:, :], scalar=tb[:, :],
                        in1=g[:, :], op0=Alu.is_lt, op1=Alu.mult)

    nc.compile()
    return nc


def _get_nc():
    if "nc" not in _CACHED:
        _CACHED["nc"] = _build_nc()
    return _CACHED["nc"]


def _decode_flat_idx(flat):
    """ucode flat index within an expert row -> global token id.

    Partition p (of 16): p < 8 -> core p, token offset o; p >= 8 ->
    core p-8, token offset VPP + o (the TAIL chunk).
    """
    p = flat // VPP
    o = flat - p * VPP
    core = np.where(p < 8, p, p - 8)
    base = np.where(p < 8, 0, VPP)
    return TSH * core + base + o


def make_in_maps(x, W, b):
    x = np.ascontiguousarray(np.asarray(x), dtype=np.float32)
    W = np.ascontiguousarray(np.asarray(W), dtype=np.float32)
    b = np.ascontiguousarray(np.asarray(b), dtype=np.float32).reshape(E, 1)
    assert x.shape == (T, D) and W.shape == (E, D)
    ident = np.eye(128, dtype=np.float32)
    return [
        {"x": np.ascontiguousarray(x[TSH * c:TSH * (c + 1)].T), "W": W,
         "b": b, "ident": ident}
        for c in range(NCORES)
    ]


def kernel(x, W, b, k):
    from concourse.bass_utils import run_bass_kernel_spmd

    assert int(k) == K, f"kernel compiled for k={K}, got {k}"
    in_maps = make_in_maps(x, W, b)
    nc = _get_nc()
    res = run_bass_kernel_spmd(nc, in_maps, core_ids=list(range(NCORES)))

    vals = np.empty((E, K), dtype=np.float32)
    idx = np.empty((E, K), dtype=np.int32)
    for c in range(NCORES):
        raw = res.results[c]["raw"]          # [ROUNDS, 128, 32] uint32
        for el in range(EPC):
            e = EPC * c + el
            blk = raw[:, 16 * el:16 * el + 16, :]        # [ROUNDS, 16, 32]
            v = blk[:, :, :16].reshape(ROUNDS, 256).view(np.float32)[:, ::-1]
            fi = blk[:, :, 16:32].reshape(ROUNDS, 256)[:, ::-1]
            vals[e] = v.reshape(K)
            idx[e] = _decode_flat_idx(fi.astype(np.int64)).reshape(K)
    return vals, idx
